# revision 3
# baseline (speedup 1.0000x reference)
"""DoubleFeatureTransformerSlice — Trainium2 Bass kernel.

out_s[b, :] = bias + sum_k values_s[b, k] * weight[indices_s[b, k], :]   (s = 0, 1)

Sharding: data-parallel over batch across 8 NeuronCores; weight replicated.
Each core handles 1024 rows of slice0 + 1024 rows of slice1 (16 tiles of 128
samples).  No collectives; outputs concatenated on host.

MODE "fp8" (shipped): weight table quantized host-side to fp8 e3m4
(w * 2^11, sigma*2^11 = 13.6 < 15.5 = e3m4 max; 4 mantissa bits suit the
uniform weight distribution).  Per tile of 128 samples, 4 dma_gather calls
(queue round-robin over 2 SWDGE queues) each pull 1024 rows of 1 KB fp8.
PE accumulates psum += diag(v_k)[fp16] @ rows_k[fp8e3] in fp32 PSUM (mixed
fp16 x fp8 matmul streams at the fp8 rate); DVE epilogue rescales by 2^-11
and adds bias: out = psum * scl + bias.

Accuracy: host-emulated and HW-measured rel err (max|err| / max|expected|)
= 1.315e-2 — quantization is entirely host-side; the device only does exact
fp8 x fp16 products into fp32 PSUM, so HW matches the host emulation.
Gate is 2e-2.

Measured (repeat-slope, interleaved R=1/R=9 dispatches, 8 cores):
  f32  exact (rel 3.2e-7): 597 us — SBUF-fabric byte-bound (256 MB/core).
  fp16 (rel 2.9e-4):       278 us — byte-bound (128 MB/core).
  fp8  e3m4 (rel 1.3e-2):  ~145 us — byte floor for 64 MB/core of gathers.
Key HW facts found on the way: SWDGE indirect_dma_start costs ~1 us of Pool
descriptor-generation per call (512 calls -> gen-bound at ~490 us for fp16),
while dma_gather amortizes gen over 1024 rows/call; a single SWDGE queue
caps 1-KB-row gathers at ~305 GB/s (per-queue descriptor rate), two queues
reach the ~440 GB/s fabric/byte ceiling; >1024 descriptors per dma_gather
call (gpg>8) overflows the SWDGE ring carveout and wedges the device.
"""

import numpy as np

MODE = "fp8"  # "fp8" | "f32"

NCORES = 8
B = 8192
K = 32
D = 1024
V = 22528
P = 128
BPC = B // NCORES          # batch rows per core per slice
ROWS = 2 * BPC             # rows per core (slice0 chunk + slice1 chunk)
NTILES = ROWS // P         # 16 tiles of 128 samples

# fp8 mode tuning (A/B'd on HW)
FP8_GPG = 8                # k-values per dma_gather call (1024 rows = ring max)
FP8_GATH_BUFS = 10
FP8_NQUEUES = 2
FP8_LOG2_SCALE = 11        # w * 2^11: sigma*2^11 = 13.6 < 15.5 (e3m4 max)

_cached = {}
LAST_RESULTS = None        # BassKernelResults of the last run (for harness)


def _build_fp8(repeats: int = 1, gpg: int = FP8_GPG, gath_bufs: int = FP8_GATH_BUFS,
               nqueues: int = FP8_NQUEUES, diag_bufs: int = 6, out_bufs: int = 3):
    import concourse.bacc as bacc
    import concourse.mybir as mybir
    import concourse.tile as tile
    from concourse.masks import make_identity

    nidx = gpg * P                 # rows per dma_gather call
    cpg = nidx // 16               # idx16 columns per call
    ngath = NTILES * (K // gpg)    # calls per core

    nc = bacc.Bacc(
        "TRN2",
        target_bir_lowering=False,
        debug=False,
        enable_asserts=False,
        num_devices=NCORES,
        num_swdge_queues=max(nqueues, 1),
    )
    w = nc.dram_tensor("w", [V, D], mybir.dt.float8e3, kind="ExternalInput")
    idx16 = nc.dram_tensor("idx16", [P, ngath * cpg], mybir.dt.int16,
                           kind="ExternalInput")
    val = nc.dram_tensor("val", [ROWS, K], mybir.dt.float32, kind="ExternalInput")
    bias = nc.dram_tensor("bias_bcast", [P, D], mybir.dt.float32,
                          kind="ExternalInput")
    scl = nc.dram_tensor("scl", [P, 1], mybir.dt.float32, kind="ExternalInput")
    out = nc.dram_tensor("out", [ROWS, D], mybir.dt.float32, kind="ExternalOutput")

    with tile.TileContext(nc) as tc:
        with (
            tc.tile_pool(name="gath", bufs=gath_bufs) as gpool,
            tc.tile_pool(name="diag", bufs=diag_bufs) as dpool,
            tc.tile_pool(name="psum", bufs=2, space="PSUM") as ppool,
            tc.tile_pool(name="outs", bufs=out_bufs) as opool,
            tc.tile_pool(name="const", bufs=1) as cpool,
        ):
            ident = cpool.tile([P, P], mybir.dt.float16, tag="ident")
            make_identity(nc, ident[:])
            bias_t = cpool.tile([P, D], mybir.dt.float32, tag="bias")
            nc.sync.dma_start(bias_t[:], bias[:, :])
            scl_t = cpool.tile([P, 1], mybir.dt.float32, tag="scl")
            nc.sync.dma_start(scl_t[:], scl[:, :])
            idxs = cpool.tile([P, ngath * cpg], mybir.dt.int16, tag="idxs")
            nc.sync.dma_start(idxs[:], idx16[:, :])
            val_all = cpool.tile([P, NTILES, K], mybir.dt.float32, tag="vala")
            nc.sync.dma_start(val_all[:], val[:, :].rearrange("(t p) k -> p t k", p=P))
            call = 0
            for t in range(NTILES * repeats):
                t = t % NTILES
                r0 = t * P
                psum = ppool.tile([P, D], mybir.dt.float32, tag="ps")
                for gi in range(K // gpg):
                    gid = t * (K // gpg) + gi
                    g = gpool.tile([P, gpg, D], mybir.dt.float8e3, tag="g")
                    nc.gpsimd.dma_gather(
                        g[:],
                        w[:, :],
                        idxs[:, gid * cpg : (gid + 1) * cpg],
                        nidx,
                        nidx,
                        D,
                        queue_num=call % max(nqueues, 1),
                    )
                    call += 1
                    for j in range(gpg):
                        k = gi * gpg + j
                        diag = dpool.tile([P, P], mybir.dt.float16, tag="dg")
                        nc.vector.tensor_scalar(
                            out=diag[:],
                            in0=ident[:],
                            scalar1=val_all[:, t, k : k + 1],
                            scalar2=None,
                            op0=mybir.AluOpType.mult,
                        )
                        first, last = k == 0, k == K - 1
                        for h in range(2):  # psum halves live in separate banks
                            nc.tensor.matmul(
                                out=psum[:, h * 512 : (h + 1) * 512],
                                lhsT=diag[:],
                                rhs=g[:, j, h * 512 : (h + 1) * 512],
                                start=first,
                                stop=last,
                            )
                outt = opool.tile([P, D], mybir.dt.float32, tag="o")
                nc.vector.scalar_tensor_tensor(
                    out=outt[:],
                    in0=psum[:],
                    scalar=scl_t[:, 0:1],
                    in1=bias_t[:],
                    op0=mybir.AluOpType.mult,
                    op1=mybir.AluOpType.add,
                )
                nc.sync.dma_start(out[r0 : r0 + P, :], outt[:])
    nc.compile()
    return nc


def _build_f32(repeats: int = 1, gath_bufs: int = 32, accp_bufs: int = 6):
    """Exact-f32 fallback (rel err ~3e-7, ~597 us): per (tile, k) one SWDGE
    indirect DMA gathers 128 weight rows (4 KB f32); DVE scalar_tensor_tensor
    does acc = gathered * v[:, k] + acc (k=0 reads broadcast bias)."""
    import concourse.bacc as bacc
    import concourse.bass as bass
    import concourse.mybir as mybir
    import concourse.tile as tile

    nc = bacc.Bacc(
        "TRN2",
        target_bir_lowering=False,
        debug=False,
        enable_asserts=False,
        num_devices=NCORES,
    )
    w = nc.dram_tensor("w", [V, D], mybir.dt.float32, kind="ExternalInput")
    idx = nc.dram_tensor("idx", [ROWS, K], mybir.dt.int32, kind="ExternalInput")
    val = nc.dram_tensor("val", [ROWS, K], mybir.dt.float32, kind="ExternalInput")
    bias = nc.dram_tensor("bias_bcast", [P, D], mybir.dt.float32, kind="ExternalInput")
    out = nc.dram_tensor("out", [ROWS, D], mybir.dt.float32, kind="ExternalOutput")

    with tile.TileContext(nc) as tc:
        with (
            tc.tile_pool(name="gath", bufs=gath_bufs) as gpool,
            tc.tile_pool(name="accp", bufs=accp_bufs) as apool,
            tc.tile_pool(name="const", bufs=1) as cpool,
        ):
            bias_t = cpool.tile([P, D], mybir.dt.float32)
            nc.sync.dma_start(bias_t[:], bias[:, :])
            idx_all = cpool.tile([P, NTILES, K], mybir.dt.int32, tag="idxa")
            val_all = cpool.tile([P, NTILES, K], mybir.dt.float32, tag="vala")
            nc.sync.dma_start(idx_all[:], idx[:, :].rearrange("(t p) k -> p t k", p=P))
            nc.sync.dma_start(val_all[:], val[:, :].rearrange("(t p) k -> p t k", p=P))
            for t in range(NTILES * repeats):
                t = t % NTILES
                r0 = t * P
                acc = apool.tile([P, D], mybir.dt.float32, tag="acc")
                for k in range(K):
                    g = gpool.tile([P, D], mybir.dt.float32, tag="g")
                    nc.gpsimd.indirect_dma_start(
                        out=g[:],
                        out_offset=None,
                        in_=w[:, :],
                        in_offset=bass.IndirectOffsetOnAxis(
                            ap=idx_all[:, t, k : k + 1], axis=0
                        ),
                    )
                    nc.vector.scalar_tensor_tensor(
                        out=acc[:],
                        in0=g[:],
                        scalar=val_all[:, t, k : k + 1],
                        in1=(bias_t[:] if k == 0 else acc[:]),
                        op0=mybir.AluOpType.mult,
                        op1=mybir.AluOpType.add,
                    )
                nc.sync.dma_start(out[r0 : r0 + P, :], acc[:])
    nc.compile()
    return nc


def _build(repeats: int = 1, mode: str | None = None):
    mode = mode or MODE
    if mode == "f32":
        return _build_f32(repeats)
    return _build_fp8(repeats)


def _wrap_idx16(idx_c: np.ndarray, gpg: int = FP8_GPG) -> np.ndarray:
    """[ROWS, K] int -> [P, ngath * nidx/16] int16 in dma_gather's wrap-16
    layout (index i of a gather lives at [i % 16, i // 16]; pattern replicated
    across all 128 partitions)."""
    nidx = gpg * P
    A = idx_c.reshape(NTILES, P, K // gpg, gpg)  # [t, p, gi, j]
    cols = []
    for t in range(NTILES):
        for gi in range(K // gpg):
            flat = A[t, :, gi, :].T.reshape(-1)          # i = j*128 + p
            cols.append(flat.reshape(nidx // 16, 16).T)  # [16, cpg]
    w16 = np.concatenate(cols, axis=1)
    return np.ascontiguousarray(np.tile(w16, (P // 16, 1)).astype(np.int16))


def prep_in_maps(fi0, fv0, fi1, fv1, weight, bias, mode=None):
    mode = mode or MODE
    b = np.asarray(bias, dtype=np.float32)
    bias_b = np.ascontiguousarray(np.broadcast_to(b[None, :], (P, D)))
    wf = np.asarray(weight, dtype=np.float32)
    if mode == "f32":
        w = np.ascontiguousarray(wf)
    else:
        import ml_dtypes

        w = np.ascontiguousarray(
            (wf * np.float32(2.0 ** FP8_LOG2_SCALE)).astype(ml_dtypes.float8_e3m4)
        )
    scl = np.full((P, 1), 2.0 ** -FP8_LOG2_SCALE, dtype=np.float32)
    in_maps = []
    for c in range(NCORES):
        sl = slice(c * BPC, (c + 1) * BPC)
        idx_c = np.concatenate([fi0[sl], fi1[sl]], axis=0)
        val_c = np.ascontiguousarray(
            np.concatenate([fv0[sl], fv1[sl]], axis=0).astype(np.float32)
        )
        m = {"w": w, "val": val_c, "bias_bcast": bias_b}
        if mode == "f32":
            m["idx"] = np.ascontiguousarray(idx_c.astype(np.int32))
        else:
            m["idx16"] = _wrap_idx16(idx_c)
            m["scl"] = scl
        in_maps.append(m)
    return in_maps


def kernel(
    feature_indices_0,
    feature_values_0,
    feature_indices_1,
    feature_values_1,
    weight,
    bias,
):
    global LAST_RESULTS
    from concourse.bass_utils import run_bass_kernel_spmd

    if MODE not in _cached:
        _cached[MODE] = _build(mode=MODE)
    nc = _cached[MODE]

    in_maps = prep_in_maps(
        np.asarray(feature_indices_0),
        np.asarray(feature_values_0),
        np.asarray(feature_indices_1),
        np.asarray(feature_values_1),
        weight,
        bias,
        MODE,
    )
    try:
        res = run_bass_kernel_spmd(nc, in_maps, core_ids=list(range(NCORES)))
    except ModuleNotFoundError:
        # BASS_TRACE set but this axon client lacks the NTFF profile hook
        # (antenv.axon_hooks) — rerun with tracing disabled.
        import os

        os.environ["BASS_NEVER_TRACE"] = "1"
        res = run_bass_kernel_spmd(nc, in_maps, core_ids=list(range(NCORES)))
    LAST_RESULTS = res
    outs = [r["out"] for r in res.results]
    out0 = np.concatenate([o[:BPC] for o in outs], axis=0)
    out1 = np.concatenate([o[BPC:] for o in outs], axis=0)
    return (out0, out1)


# revision 4
# speedup vs baseline: 5.4999x; 5.4999x over previous
"""DoubleFeatureTransformerSlice — Trainium2 Bass kernel.

out_s[b, :] = bias + sum_k values_s[b, k] * weight[indices_s[b, k], :]   (s = 0, 1)

Sharding: data-parallel over batch across 8 NeuronCores; weight replicated.
Each core handles 1024 rows of slice0 + 1024 rows of slice1 (16 tiles of 128
samples).  No collectives; outputs concatenated on host.

MODE "fp8" (shipped): weight table quantized host-side to fp8 e3m4
(w * 2^11, sigma*2^11 = 13.6 < 15.5 = e3m4 max; 4 mantissa bits suit the
uniform weight distribution).  Per tile of 128 samples, 4 dma_gather calls
(queue round-robin over 2 SWDGE queues) each pull 1024 rows of 1 KB fp8.
PE accumulates psum += diag(v_k)[fp16] @ rows_k[fp8e3] in fp32 PSUM (mixed
fp16 x fp8 matmul streams at the fp8 rate); DVE epilogue rescales by 2^-11
and adds bias: out = psum * scl + bias.

Accuracy: host-emulated and HW-measured rel err (max|err| / max|expected|)
= 1.315e-2 — quantization is entirely host-side; the device only does exact
fp8 x fp16 products into fp32 PSUM, so HW matches the host emulation.
Gate is 2e-2.

Measured (repeat-slope, interleaved R=1/R=9 dispatches, 8 cores):
  f32  exact (rel 3.2e-7): 597 us — SBUF-fabric byte-bound (256 MB/core).
  fp16 (rel 2.9e-4):       278 us — byte-bound (128 MB/core).
  fp8  e3m4 (rel 1.3e-2):  ~145 us — byte floor for 64 MB/core of gathers.
Key HW facts found on the way: SWDGE indirect_dma_start costs ~1 us of Pool
descriptor-generation per call (512 calls -> gen-bound at ~490 us for fp16),
while dma_gather amortizes gen over 1024 rows/call; a single SWDGE queue
caps 1-KB-row gathers at ~305 GB/s (per-queue descriptor rate), two queues
reach the ~440 GB/s fabric/byte ceiling; >1024 descriptors per dma_gather
call (gpg>8) overflows the SWDGE ring carveout and wedges the device.
"""

import numpy as np

MODE = "fp8"  # "fp8" | "f32"

NCORES = 8
B = 8192
K = 32
D = 1024
V = 22528
P = 128
BPC = B // NCORES          # batch rows per core per slice
ROWS = 2 * BPC             # rows per core (slice0 chunk + slice1 chunk)
NTILES = ROWS // P         # 16 tiles of 128 samples

# fp8 mode tuning (A/B'd on HW)
FP8_GPG = 8                # k-values per dma_gather call (1024 rows = ring max)
FP8_GATH_BUFS = 10
FP8_NQUEUES = 2
FP8_LOG2_SCALE = 11        # w * 2^11: sigma*2^11 = 13.6 < 15.5 (e3m4 max)

_cached = {}
LAST_RESULTS = None        # BassKernelResults of the last run (for harness)


def _build_fp8(repeats: int = 1, gpg: int = FP8_GPG, gath_bufs: int = FP8_GATH_BUFS,
               nqueues: int = FP8_NQUEUES, diag_bufs: int = 12, out_bufs: int = 4):
    import concourse.bacc as bacc
    import concourse.mybir as mybir
    import concourse.tile as tile
    from concourse.masks import make_identity

    nidx = gpg * P                 # rows per dma_gather call
    cpg = nidx // 16               # idx16 columns per call
    ngath = NTILES * (K // gpg)    # calls per core

    nc = bacc.Bacc(
        "TRN2",
        target_bir_lowering=False,
        debug=False,
        enable_asserts=False,
        num_devices=NCORES,
        num_swdge_queues=max(nqueues, 1),
    )
    w = nc.dram_tensor("w", [V, D], mybir.dt.float8e3, kind="ExternalInput")
    idx16 = nc.dram_tensor("idx16", [P, ngath * cpg], mybir.dt.int16,
                           kind="ExternalInput")
    val = nc.dram_tensor("val", [ROWS, K], mybir.dt.float32, kind="ExternalInput")
    bias = nc.dram_tensor("bias_bcast", [P, D], mybir.dt.float32,
                          kind="ExternalInput")
    scl = nc.dram_tensor("scl", [P, 1], mybir.dt.float32, kind="ExternalInput")
    out = nc.dram_tensor("out", [ROWS, D], mybir.dt.float32, kind="ExternalOutput")

    with tile.TileContext(nc) as tc:
        with (
            tc.tile_pool(name="gath", bufs=gath_bufs) as gpool,
            tc.tile_pool(name="diag", bufs=diag_bufs) as dpool,
            tc.tile_pool(name="psum", bufs=4, space="PSUM") as ppool,
            tc.tile_pool(name="outs", bufs=out_bufs) as opool,
            tc.tile_pool(name="const", bufs=1) as cpool,
        ):
            ident = cpool.tile([P, P], mybir.dt.float16, tag="ident")
            make_identity(nc, ident[:])
            bias_t = cpool.tile([P, D], mybir.dt.float32, tag="bias")
            nc.sync.dma_start(bias_t[:], bias[:, :])
            scl_t = cpool.tile([P, 1], mybir.dt.float32, tag="scl")
            nc.sync.dma_start(scl_t[:], scl[:, :])
            idxs = cpool.tile([P, ngath * cpg], mybir.dt.int16, tag="idxs")
            nc.sync.dma_start(idxs[:], idx16[:, :])
            val_all = cpool.tile([P, NTILES, K], mybir.dt.float32, tag="vala")
            nc.sync.dma_start(val_all[:], val[:, :].rearrange("(t p) k -> p t k", p=P))
            call = 0
            for t in range(NTILES * repeats):
                t = t % NTILES
                r0 = t * P
                psum = ppool.tile([P, D], mybir.dt.float32, tag="ps")
                for gi in range(K // gpg):
                    gid = t * (K // gpg) + gi
                    g = gpool.tile([P, gpg, D], mybir.dt.float8e3, tag="g")
                    nc.gpsimd.dma_gather(
                        g[:],
                        w[:, :],
                        idxs[:, gid * cpg : (gid + 1) * cpg],
                        nidx,
                        nidx,
                        D,
                        queue_num=call % max(nqueues, 1),
                    )
                    call += 1
                    for j in range(gpg):
                        k = gi * gpg + j
                        diag = dpool.tile([P, P], mybir.dt.float16, tag="dg")
                        nc.vector.tensor_scalar(
                            out=diag[:],
                            in0=ident[:],
                            scalar1=val_all[:, t, k : k + 1],
                            scalar2=None,
                            op0=mybir.AluOpType.mult,
                        )
                        first, last = k == 0, k == K - 1
                        for h in range(2):  # psum halves live in separate banks
                            nc.tensor.matmul(
                                out=psum[:, h * 512 : (h + 1) * 512],
                                lhsT=diag[:],
                                rhs=g[:, j, h * 512 : (h + 1) * 512],
                                start=first,
                                stop=last,
                            )
                outt = opool.tile([P, D], mybir.dt.float32, tag="o")
                nc.vector.scalar_tensor_tensor(
                    out=outt[:],
                    in0=psum[:],
                    scalar=scl_t[:, 0:1],
                    in1=bias_t[:],
                    op0=mybir.AluOpType.mult,
                    op1=mybir.AluOpType.add,
                )
                nc.sync.dma_start(out[r0 : r0 + P, :], outt[:])
    nc.compile()
    return nc


def _build_f32(repeats: int = 1, gath_bufs: int = 32, accp_bufs: int = 6):
    """Exact-f32 fallback (rel err ~3e-7, ~597 us): per (tile, k) one SWDGE
    indirect DMA gathers 128 weight rows (4 KB f32); DVE scalar_tensor_tensor
    does acc = gathered * v[:, k] + acc (k=0 reads broadcast bias)."""
    import concourse.bacc as bacc
    import concourse.bass as bass
    import concourse.mybir as mybir
    import concourse.tile as tile

    nc = bacc.Bacc(
        "TRN2",
        target_bir_lowering=False,
        debug=False,
        enable_asserts=False,
        num_devices=NCORES,
    )
    w = nc.dram_tensor("w", [V, D], mybir.dt.float32, kind="ExternalInput")
    idx = nc.dram_tensor("idx", [ROWS, K], mybir.dt.int32, kind="ExternalInput")
    val = nc.dram_tensor("val", [ROWS, K], mybir.dt.float32, kind="ExternalInput")
    bias = nc.dram_tensor("bias_bcast", [P, D], mybir.dt.float32, kind="ExternalInput")
    out = nc.dram_tensor("out", [ROWS, D], mybir.dt.float32, kind="ExternalOutput")

    with tile.TileContext(nc) as tc:
        with (
            tc.tile_pool(name="gath", bufs=gath_bufs) as gpool,
            tc.tile_pool(name="accp", bufs=accp_bufs) as apool,
            tc.tile_pool(name="const", bufs=1) as cpool,
        ):
            bias_t = cpool.tile([P, D], mybir.dt.float32)
            nc.sync.dma_start(bias_t[:], bias[:, :])
            idx_all = cpool.tile([P, NTILES, K], mybir.dt.int32, tag="idxa")
            val_all = cpool.tile([P, NTILES, K], mybir.dt.float32, tag="vala")
            nc.sync.dma_start(idx_all[:], idx[:, :].rearrange("(t p) k -> p t k", p=P))
            nc.sync.dma_start(val_all[:], val[:, :].rearrange("(t p) k -> p t k", p=P))
            for t in range(NTILES * repeats):
                t = t % NTILES
                r0 = t * P
                acc = apool.tile([P, D], mybir.dt.float32, tag="acc")
                for k in range(K):
                    g = gpool.tile([P, D], mybir.dt.float32, tag="g")
                    nc.gpsimd.indirect_dma_start(
                        out=g[:],
                        out_offset=None,
                        in_=w[:, :],
                        in_offset=bass.IndirectOffsetOnAxis(
                            ap=idx_all[:, t, k : k + 1], axis=0
                        ),
                    )
                    nc.vector.scalar_tensor_tensor(
                        out=acc[:],
                        in0=g[:],
                        scalar=val_all[:, t, k : k + 1],
                        in1=(bias_t[:] if k == 0 else acc[:]),
                        op0=mybir.AluOpType.mult,
                        op1=mybir.AluOpType.add,
                    )
                nc.sync.dma_start(out[r0 : r0 + P, :], acc[:])
    nc.compile()
    return nc


def _build(repeats: int = 1, mode: str | None = None):
    mode = mode or MODE
    if mode == "f32":
        return _build_f32(repeats)
    return _build_fp8(repeats)


def _wrap_idx16(idx_c: np.ndarray, gpg: int = FP8_GPG) -> np.ndarray:
    """[ROWS, K] int -> [P, ngath * nidx/16] int16 in dma_gather's wrap-16
    layout (index i of a gather lives at [i % 16, i // 16]; pattern replicated
    across all 128 partitions)."""
    nidx = gpg * P
    A = idx_c.reshape(NTILES, P, K // gpg, gpg)  # [t, p, gi, j]
    cols = []
    for t in range(NTILES):
        for gi in range(K // gpg):
            flat = A[t, :, gi, :].T.reshape(-1)          # i = j*128 + p
            cols.append(flat.reshape(nidx // 16, 16).T)  # [16, cpg]
    w16 = np.concatenate(cols, axis=1)
    return np.ascontiguousarray(np.tile(w16, (P // 16, 1)).astype(np.int16))


def prep_in_maps(fi0, fv0, fi1, fv1, weight, bias, mode=None):
    mode = mode or MODE
    b = np.asarray(bias, dtype=np.float32)
    bias_b = np.ascontiguousarray(np.broadcast_to(b[None, :], (P, D)))
    wf = np.asarray(weight, dtype=np.float32)
    if mode == "f32":
        w = np.ascontiguousarray(wf)
    else:
        import ml_dtypes

        w = np.ascontiguousarray(
            (wf * np.float32(2.0 ** FP8_LOG2_SCALE)).astype(ml_dtypes.float8_e3m4)
        )
    scl = np.full((P, 1), 2.0 ** -FP8_LOG2_SCALE, dtype=np.float32)
    in_maps = []
    for c in range(NCORES):
        sl = slice(c * BPC, (c + 1) * BPC)
        idx_c = np.concatenate([fi0[sl], fi1[sl]], axis=0)
        val_c = np.ascontiguousarray(
            np.concatenate([fv0[sl], fv1[sl]], axis=0).astype(np.float32)
        )
        m = {"w": w, "val": val_c, "bias_bcast": bias_b}
        if mode == "f32":
            m["idx"] = np.ascontiguousarray(idx_c.astype(np.int32))
        else:
            m["idx16"] = _wrap_idx16(idx_c)
            m["scl"] = scl
        in_maps.append(m)
    return in_maps


def kernel(
    feature_indices_0,
    feature_values_0,
    feature_indices_1,
    feature_values_1,
    weight,
    bias,
):
    global LAST_RESULTS
    from concourse.bass_utils import run_bass_kernel_spmd

    if MODE not in _cached:
        _cached[MODE] = _build(mode=MODE)
    nc = _cached[MODE]

    in_maps = prep_in_maps(
        np.asarray(feature_indices_0),
        np.asarray(feature_values_0),
        np.asarray(feature_indices_1),
        np.asarray(feature_values_1),
        weight,
        bias,
        MODE,
    )
    try:
        res = run_bass_kernel_spmd(nc, in_maps, core_ids=list(range(NCORES)))
    except ModuleNotFoundError:
        # BASS_TRACE set but this axon client lacks the NTFF profile hook
        # (antenv.axon_hooks) — rerun with tracing disabled.
        import os

        os.environ["BASS_NEVER_TRACE"] = "1"
        res = run_bass_kernel_spmd(nc, in_maps, core_ids=list(range(NCORES)))
    LAST_RESULTS = res
    outs = [r["out"] for r in res.results]
    out0 = np.concatenate([o[:BPC] for o in outs], axis=0)
    out1 = np.concatenate([o[BPC:] for o in outs], axis=0)
    return (out0, out1)
